# revision 10
# baseline (speedup 1.0000x reference)
"""GraphSAGE layer on 8 Trainium2 NeuronCores.

Strategy: sort edges by receiver on host; shard receivers across the 8 cores
(6250 each). Per core the segment-sum is computed per 128-receiver block by
gathering sender features and accumulating feat^T @ onehot into PSUM, giving
the mean-aggregated features pre-transposed ([feat, recv]) as lhsT for the
fused Dense layer.

v2: all feature data in bf16. Sender-feature gathers use batched
`dma_gather` (one instruction per chunk+sender-half instead of one indirect
DMA per 128 edges) — dma_gather takes int16 indices, so edges are split per
block into low (snd < 32768) / high columns and gathered from base/offset
views of the node table. Matmuls run in bf16 (1 cycle/row vs 4 for fp32),
one-hots are built on DVE in bf16, ReLU runs on the idle scalar engine, and
outputs store as bf16 (upcast on host).
"""
import numpy as np
import ml_dtypes
import concourse.bass as bass
import concourse.tile as tile
from concourse import mybir
from concourse import library_config
from concourse.bass_utils import run_bass_kernel_spmd

f32 = mybir.dt.float32
bf16 = mybir.dt.bfloat16
i16 = mybir.dt.int16

N, D, H = 50000, 128, 256
NCORES = 8
R = N // NCORES          # receivers per core
BLK = 128                # receivers per block
NBLK = (R + BLK - 1) // BLK  # 49
RPAD = NBLK * BLK        # 6272
HALF = 32768             # int16 index limit for dma_gather
CHUNK = 7                # receiver blocks per gather chunk
NBF = np.dtype(ml_dtypes.bfloat16)


def _split_excess_waits(nc, max_waits=1, nop_waits=1):
    """walrus in this toolchain accepts only one sem-wait per instruction;
    hoist extra waits onto same-engine nops placed just before."""
    for bb in nc.main_func.blocks:
        insts = list(bb.instructions)
        new_list = []
        for ins in insts:
            si = ins.sync_info
            waits = list(si.on_wait) if (si is not None and si.on_wait) else []
            if len(waits) > max_waits:
                extra, keep = waits[:-max_waits], waits[-max_waits:]
                for k in range(0, len(extra), nop_waits):
                    nop = mybir.InstNoOp(
                        name=nc.get_next_instruction_name(), ins=[], outs=[]
                    )
                    nop.engine = ins.engine
                    nop.sync_info = mybir.SyncInfo(
                        on_wait=extra[k : k + nop_waits], on_update=[]
                    )
                    nc.register_instruction(nop, overwrite=True)
                    new_list.append(nop)
                si.on_wait = keep
            new_list.append(ins)
        bb.instructions[:] = new_list


_prog_cache = {}


def _build_program(Tlow, Thigh, with_bias, rep=1, do_gather=True, do_compute=True, sub_cols=0):
    NT = int(Tlow.sum() + Thigh.sum())
    chunks = []  # (col_base, [(j, lo_cols, hi_cols)...], ncols_low, ncols_high)
    base = 0
    for c0 in range(0, NBLK, CHUNK):
        js = list(range(c0, min(c0 + CHUNK, NBLK)))
        lo = int(sum(Tlow[j] for j in js))
        hi = int(sum(Thigh[j] for j in js))
        chunks.append((base, js, lo, hi))
        base += lo + hi

    nc = bass.Bass(target_bir_lowering=False, num_swdge_queues=4)
    nodes_d = nc.dram_tensor("nodes", [N, D], bf16, kind="ExternalInput")
    idx_d = nc.dram_tensor("idx", [128, NT * 8], i16, kind="ExternalInput")
    ridT_d = nc.dram_tensor("ridT", [128, NT], f32, kind="ExternalInput")
    invcT_d = nc.dram_tensor("invcT", [128, NT], f32, kind="ExternalInput")
    selfT_d = nc.dram_tensor("selfT", [128, RPAD], bf16, kind="ExternalInput")
    wt_d = nc.dram_tensor("wt", [128, H], bf16, kind="ExternalInput")
    wb_d = nc.dram_tensor("wb", [128, H], bf16, kind="ExternalInput")
    if with_bias:
        bias_d = nc.dram_tensor("bias", [1, H], bf16, kind="ExternalInput")
    # [p, j, h] layout: out row j*BLK+p lives at out_d[p, j, :]; host
    # transposes back. Lets groups of blocks go out in one dma_start.
    out_d = nc.dram_tensor("out", [128, NBLK, H], bf16, kind="ExternalOutput")

    with tile.TileContext(nc) as tc:
        with (
            tc.tile_pool(name="const", bufs=1) as constp,
            tc.tile_pool(name="gat", bufs=4) as gatp,
            tc.tile_pool(name="ohp", bufs=8) as ohp,
            tc.tile_pool(name="meanp", bufs=3) as meanp,
            tc.tile_pool(name="outp", bufs=3) as outp,
            tc.tile_pool(name="pmean", bufs=3, space="PSUM") as pmean,
            tc.tile_pool(name="pout", bufs=4, space="PSUM") as pout,
        ):
            iota_s = constp.tile([128, BLK], bf16)
            nc.gpsimd.iota(
                iota_s[:], [[1, BLK]], channel_multiplier=0,
                allow_small_or_imprecise_dtypes=True,
            )
            nc.gpsimd.load_library(library_config.mlp)
            _regs = {}

            def _nreg(v):
                if v not in _regs:
                    _regs[v] = nc.gpsimd.to_reg(v)
                return _regs[v]
            # load order: gather-critical tables first, selfT (big, needed
            # only at the dense stage) last
            idx_s = constp.tile([128, NT * 8], i16)
            nc.sync.dma_start(idx_s[:], idx_d[:])
            ridT_s = constp.tile([128, NT], f32)
            nc.sync.dma_start(ridT_s[:], ridT_d[:])
            invcT_s = constp.tile([128, NT], f32)
            nc.sync.dma_start(invcT_s[:], invcT_d[:])
            wt_s = constp.tile([128, H], bf16)
            nc.sync.dma_start(wt_s[:], wt_d[:])
            wb_s = constp.tile([128, H], bf16)
            nc.sync.dma_start(wb_s[:], wb_d[:])
            if with_bias:
                ones_s = constp.tile([1, 128], bf16)
                nc.vector.memset(ones_s[:], 1.0)
                bias_s = constp.tile([1, H], bf16)
                nc.sync.dma_start(bias_s[:], bias_d[:])
            selfT_s = constp.tile([128, RPAD], bf16)
            nc.sync.dma_start(selfT_s[:], selfT_d[:])

            qrr = [0]
            for _rep in range(rep):
              for base, js, lo, hi in chunks:
                  ncols = lo + hi
                  g = gatp.tile([128, ncols, D], bf16, tag="g")
                  def _emit_gather(c0, cn, src):
                      step = cn if sub_cols == 0 else sub_cols
                      for s0 in range(0, cn, step):
                          sn = min(step, cn - s0)
                          nc.gpsimd.dma_gather(
                              out_ap=g[:, c0 + s0 : c0 + s0 + sn, :],
                              in_ap=src,
                              idxs_ap=idx_s[
                                  :, (base + c0 + s0) * 8 : (base + c0 + s0 + sn) * 8
                              ],
                              num_idxs=sn * 128,
                              num_idxs_reg=_nreg(sn * 128),
                              elem_size=D,
                              single_packet=(sub_cols > 0 and sub_cols <= 8),
                              queue_num=qrr[0] % 4,
                          )
                          qrr[0] += 1

                  if do_gather and lo:
                      _emit_gather(0, lo, nodes_d[:HALF, :])
                  if do_gather and hi:
                      _emit_gather(lo, hi, nodes_d[HALF:, :])
                  # per-block column ranges inside this chunk: lows first
                  lob = 0
                  hib = lo
                  if do_compute:
                      ot_g = outp.tile([128, len(js), H], bf16, tag="ot")
                  for jj, j in enumerate(js if do_compute else []):
                      tl, th = int(Tlow[j]), int(Thigh[j])
                      cols = [base + lob + t for t in range(tl)]
                      cols += [base + hib + t for t in range(th)]
                      local = [lob + t for t in range(tl)]
                      local += [hib + t for t in range(th)]
                      lob += tl
                      hib += th
                      pm = pmean.tile([128, BLK], f32)
                      for k, (col, lc) in enumerate(zip(cols, local)):
                          oh = ohp.tile([128, BLK], bf16, tag="oh")
                          nc.vector.tensor_scalar(
                              out=oh[:],
                              in0=iota_s[:],
                              scalar1=ridT_s[:, col : col + 1],
                              scalar2=invcT_s[:, col : col + 1],
                              op0=mybir.AluOpType.is_equal,
                              op1=mybir.AluOpType.mult,
                          )
                          nc.tensor.matmul(
                              out=pm[:], lhsT=g[:, lc, :], rhs=oh[:],
                              start=(k == 0), stop=(k == len(cols) - 1),
                          )
                      mean_s = meanp.tile([128, BLK], bf16)
                      nc.scalar.copy(out=mean_s[:], in_=pm[:])
                      po = pout.tile([128, H], f32)
                      nc.tensor.matmul(out=po[:], lhsT=mean_s[:], rhs=wt_s[:],
                                       start=True, stop=False)
                      nc.tensor.matmul(
                          out=po[:], lhsT=selfT_s[:, j * BLK : (j + 1) * BLK],
                          rhs=wb_s[:], start=False, stop=not with_bias,
                      )
                      if with_bias:
                          nc.tensor.matmul(out=po[:], lhsT=ones_s[:],
                                           rhs=bias_s[:], start=False, stop=True)
                      nc.scalar.activation(
                          out=ot_g[:, jj, :], in_=po[:],
                          func=mybir.ActivationFunctionType.Relu,
                      )
                  if do_compute:
                      nc.sync.dma_start(
                          out_d[:, js[0] : js[0] + len(js), :], ot_g[:]
                      )

    mybir.codegen_inst_isa_subclasses(nc)
    _split_excess_waits(nc)
    return nc


def _host_prep(nodes, senders, receivers, W, b):
    counts = np.bincount(receivers, minlength=N).astype(np.float32)
    invc_all = (1.0 / np.maximum(counts, 1.0)).astype(np.float32)

    core = receivers // R
    block = (receivers % R) // BLK
    half = (senders >= HALF).astype(np.int64)
    gkey = (core * NBLK + block) * 2 + half
    order = np.argsort(gkey, kind="stable")
    gk = gkey[order]
    ss = senders[order]
    rr = receivers[order]

    ngroups = NCORES * NBLK * 2
    cnt = np.bincount(gk, minlength=ngroups)
    gstart = np.concatenate([[0], np.cumsum(cnt)[:-1]])
    pos = np.arange(len(gk)) - gstart[gk]

    cols_need = np.ceil(cnt.reshape(NCORES, NBLK, 2) / 128.0).astype(np.int64)
    T2 = cols_need.max(axis=0)  # [NBLK, 2] program-wide columns
    # guarantee at least one column per block so PSUM is always initialized
    empty = (T2.sum(axis=1) == 0)
    T2[empty, 0] = 1
    Tlow, Thigh = T2[:, 0].copy(), T2[:, 1].copy()

    # global column base per (block, half) in chunk order: lows then highs
    col_base = np.zeros((NBLK, 2), np.int64)
    base = 0
    for c0 in range(0, NBLK, CHUNK):
        js = range(c0, min(c0 + CHUNK, NBLK))
        for j in js:
            col_base[j, 0] = base
            base += Tlow[j]
        for j in js:
            col_base[j, 1] = base
            base += Thigh[j]
    NT = int(base)

    # per-edge placement
    e_core = gk // (NBLK * 2)
    e_blockhalf = gk % (NBLK * 2)
    e_block = e_blockhalf // 2
    e_half = e_blockhalf % 2
    e_col = col_base[e_block, e_half] + pos // 128
    e_p = pos % 128

    idx_arr = np.zeros((NCORES, NT, 128), np.int16)
    rid_arr = np.full((NCORES, NT, 128), -1.0, np.float32)
    invc_arr = np.zeros((NCORES, NT, 128), np.float32)
    flat = (e_core * NT + e_col) * 128 + e_p
    idx_arr.reshape(-1)[flat] = (ss - e_half * HALF).astype(np.int16)
    rid_arr.reshape(-1)[flat] = (rr - e_core * R - e_block * BLK).astype(np.float32)
    invc_arr.reshape(-1)[flat] = invc_all[rr]

    # int16 gather table: [16, NT*8] replicated to 128 partitions;
    # entry for (col, p) at row p%16, column col*8 + p//16
    idx_tab = (
        idx_arr.reshape(NCORES, NT, 8, 16)
        .transpose(0, 3, 1, 2)
        .reshape(NCORES, 16, NT * 8)
    )
    idx_tab = np.tile(idx_tab, (1, 8, 1))  # [C, 128, NT*8]

    ridT = rid_arr.transpose(0, 2, 1)   # [C, 128, NT]
    invcT = invc_arr.transpose(0, 2, 1)

    nodes_bf = np.ascontiguousarray(nodes.astype(NBF))
    wt = np.ascontiguousarray(W[:D, :].astype(NBF))
    wb = np.ascontiguousarray(W[D:, :].astype(NBF))
    bias = np.ascontiguousarray(b.reshape(1, H).astype(NBF))
    with_bias = bool(np.any(b))

    in_maps = []
    for c in range(NCORES):
        selfT = np.zeros((D, RPAD), NBF)
        selfT[:, :R] = nodes_bf[c * R : (c + 1) * R].T
        m = {
            "nodes": nodes_bf,
            "idx": np.ascontiguousarray(idx_tab[c]),
            "ridT": np.ascontiguousarray(ridT[c]),
            "invcT": np.ascontiguousarray(invcT[c]),
            "selfT": selfT,
            "wt": wt,
            "wb": wb,
        }
        if with_bias:
            m["bias"] = bias
        in_maps.append(m)
    return Tlow, Thigh, with_bias, in_maps


def kernel(**inputs):
    nodes = np.asarray(inputs["nodes"], dtype=np.float32)
    senders = np.asarray(inputs["senders"]).astype(np.int64)
    receivers = np.asarray(inputs["receivers"]).astype(np.int64)
    W = np.asarray(inputs["W"], dtype=np.float32)
    b = np.asarray(inputs["b"], dtype=np.float32)

    Tlow, Thigh, with_bias, in_maps = _host_prep(nodes, senders, receivers, W, b)

    key = (tuple(Tlow), tuple(Thigh), with_bias)
    if key not in _prog_cache:
        _prog_cache[key] = _build_program(Tlow, Thigh, with_bias, sub_cols=16)
    nc = _prog_cache[key]

    res = run_bass_kernel_spmd(nc, in_maps, list(range(NCORES)))
    out = np.concatenate(
        [
            res.results[c]["out"]
            .astype(np.float32)
            .transpose(1, 0, 2)
            .reshape(RPAD, H)[:R]
            for c in range(NCORES)
        ],
        axis=0,
    )
    return out



# revision 29
# speedup vs baseline: 9.7054x; 9.7054x over previous
"""GraphSAGE layer on 8 Trainium2 NeuronCores.

Strategy: sort edges by receiver on host; shard receivers across the 8 cores
(6250 each). Per core the segment-sum is computed per 128-receiver block by
gathering sender features and accumulating feat^T @ onehot into PSUM, giving
the mean-aggregated features pre-transposed ([feat, recv]) as lhsT for the
fused Dense layer.

v2: all feature data in bf16. Sender-feature gathers use batched
`dma_gather` (one instruction per chunk+sender-half instead of one indirect
DMA per 128 edges) — dma_gather takes int16 indices, so edges are split per
block into low (snd < 32768) / high columns and gathered from base/offset
views of the node table. Matmuls run in bf16 (1 cycle/row vs 4 for fp32),
one-hots are built on DVE in bf16, ReLU runs on the idle scalar engine, and
outputs store as bf16 (upcast on host).
"""
import numpy as np
import ml_dtypes
import concourse.bass as bass
import concourse.tile as tile
from concourse import mybir
from concourse import library_config
from concourse.bass_utils import run_bass_kernel_spmd

f32 = mybir.dt.float32
bf16 = mybir.dt.bfloat16
i16 = mybir.dt.int16

N, D, H = 50000, 128, 256
NCORES = 8
R = N // NCORES          # receivers per core
BLK = 128                # receivers per block
NBLK = (R + BLK - 1) // BLK  # 49
RPAD = NBLK * BLK        # 6272
HALF = 32768             # int16 index limit for dma_gather
CHUNK = 7                # receiver blocks per gather chunk
NBF = np.dtype(ml_dtypes.bfloat16)


def _split_excess_waits(nc, max_waits=1, nop_waits=1):
    """walrus in this toolchain accepts only one sem-wait per instruction;
    hoist extra waits onto same-engine nops placed just before."""
    for bb in nc.main_func.blocks:
        insts = list(bb.instructions)
        new_list = []
        for ins in insts:
            si = ins.sync_info
            waits = list(si.on_wait) if (si is not None and si.on_wait) else []
            if len(waits) > max_waits:
                extra, keep = waits[:-max_waits], waits[-max_waits:]
                for k in range(0, len(extra), nop_waits):
                    nop = mybir.InstNoOp(
                        name=nc.get_next_instruction_name(), ins=[], outs=[]
                    )
                    nop.engine = ins.engine
                    nop.sync_info = mybir.SyncInfo(
                        on_wait=extra[k : k + nop_waits], on_update=[]
                    )
                    nc.register_instruction(nop, overwrite=True)
                    new_list.append(nop)
                si.on_wait = keep
            new_list.append(ins)
        bb.instructions[:] = new_list


_prog_cache = {}


def _build_program(Tlow, Thigh, with_bias, rep=1, do_gather=True, do_compute=True,
                   sub_cols=0, static_oh=False, ohp_bufs=8, pre_oh=False):
    NT = int(Tlow.sum() + Thigh.sum())
    chunks = []  # (col_base, [(j, lo_cols, hi_cols)...], ncols_low, ncols_high)
    base = 0
    for c0 in range(0, NBLK, CHUNK):
        js = list(range(c0, min(c0 + CHUNK, NBLK)))
        lo = int(sum(Tlow[j] for j in js))
        hi = int(sum(Thigh[j] for j in js))
        chunks.append((base, js, lo, hi))
        base += lo + hi

    nc = bass.Bass(target_bir_lowering=False, num_swdge_queues=4)
    nodes_d = nc.dram_tensor("nodes", [N, D], bf16, kind="ExternalInput")
    idx_d = nc.dram_tensor("idx", [128, NT * 8], i16, kind="ExternalInput")
    if pre_oh:
        # host-precomputed one-hots, [p(edge-slot), col, r] per partition
        oh_d = nc.dram_tensor("ohT", [128, NT * BLK], bf16, kind="ExternalInput")
    else:
        ridT_d = nc.dram_tensor("ridT", [128, NT], f32, kind="ExternalInput")
        invcT_d = nc.dram_tensor("invcT", [128, NT], f32, kind="ExternalInput")
    selfT_d = nc.dram_tensor("selfT", [128, RPAD], bf16, kind="ExternalInput")
    wt_d = nc.dram_tensor("wt", [128, H], bf16, kind="ExternalInput")
    wb_d = nc.dram_tensor("wb", [128, H], bf16, kind="ExternalInput")
    if with_bias:
        bias_d = nc.dram_tensor("bias", [1, H], bf16, kind="ExternalInput")
    # [p, j, h] layout: out row j*BLK+p lives at out_d[p, j, :]; host
    # transposes back. Lets groups of blocks go out in one dma_start.
    out_d = nc.dram_tensor("out", [128, NBLK, H], bf16, kind="ExternalOutput")

    with tile.TileContext(nc) as tc:
        with (
            tc.tile_pool(name="const", bufs=1) as constp,
            tc.tile_pool(name="gat", bufs=3 if pre_oh else 4) as gatp,
            tc.tile_pool(name="ohp", bufs=ohp_bufs) as ohp,
            tc.tile_pool(name="meanp", bufs=3) as meanp,
            tc.tile_pool(name="outp", bufs=3) as outp,
            tc.tile_pool(name="pmean", bufs=3, space="PSUM") as pmean,
            tc.tile_pool(name="pout", bufs=4, space="PSUM") as pout,
        ):
            iota_s = constp.tile([128, BLK], bf16)
            nc.gpsimd.iota(
                iota_s[:], [[1, BLK]], channel_multiplier=0,
                allow_small_or_imprecise_dtypes=True,
            )
            nc.gpsimd.load_library(library_config.mlp)
            _regs = {}

            def _nreg(v):
                if v not in _regs:
                    _regs[v] = nc.gpsimd.to_reg(v)
                return _regs[v]
            # load order: gather-critical tables first, selfT (big, needed
            # only at the dense stage) last
            idx_s = constp.tile([128, NT * 8], i16)
            nc.sync.dma_start(idx_s[:], idx_d[:])
            if not pre_oh:
                ridT_s = constp.tile([128, NT], f32)
                nc.sync.dma_start(ridT_s[:], ridT_d[:])
                invcT_s = constp.tile([128, NT], f32)
                nc.sync.dma_start(invcT_s[:], invcT_d[:])
            wt_s = constp.tile([128, H], bf16)
            nc.sync.dma_start(wt_s[:], wt_d[:])
            wb_s = constp.tile([128, H], bf16)
            nc.sync.dma_start(wb_s[:], wb_d[:])
            if with_bias:
                ones_s = constp.tile([1, 128], bf16)
                nc.vector.memset(ones_s[:], 1.0)
                bias_s = constp.tile([1, H], bf16)
                nc.sync.dma_start(bias_s[:], bias_d[:])
            selfT_s = constp.tile([128, RPAD], bf16)
            nc.sync.dma_start(selfT_s[:], selfT_d[:])
            if static_oh:
                oh_const = constp.tile([128, BLK], bf16)
                nc.vector.tensor_scalar(
                    out=oh_const[:], in0=iota_s[:],
                    scalar1=ridT_s[:, 0:1], scalar2=invcT_s[:, 0:1],
                    op0=mybir.AluOpType.is_equal, op1=mybir.AluOpType.mult,
                )

            qrr = [0]
            for _rep in range(rep):
              for base, js, lo, hi in chunks:
                  ncols = lo + hi
                  g = gatp.tile([128, ncols, D], bf16, tag="g")
                  def _emit_gather(c0, cn, src):
                      step = cn if sub_cols == 0 else sub_cols
                      for s0 in range(0, cn, step):
                          sn = min(step, cn - s0)
                          nc.gpsimd.dma_gather(
                              out_ap=g[:, c0 + s0 : c0 + s0 + sn, :],
                              in_ap=src,
                              idxs_ap=idx_s[
                                  :, (base + c0 + s0) * 8 : (base + c0 + s0 + sn) * 8
                              ],
                              num_idxs=sn * 128,
                              num_idxs_reg=_nreg(sn * 128),
                              elem_size=D,
                              single_packet=(sub_cols > 0 and sub_cols <= 8),
                              queue_num=qrr[0] % 4,
                          )
                          qrr[0] += 1

                  if do_gather and lo:
                      _emit_gather(0, lo, nodes_d[:HALF, :])
                  if do_gather and hi:
                      _emit_gather(lo, hi, nodes_d[N - HALF :, :])
                  if pre_oh:
                      oh_c = ohp.tile([128, ncols, BLK], bf16, tag="ohc")
                      nc.sync.dma_start(
                          oh_c[:], oh_d[:, base * BLK : (base + ncols) * BLK]
                      )
                  # per-block column ranges inside this chunk: lows first
                  lob = 0
                  hib = lo
                  if do_compute:
                      ot_g = outp.tile([128, len(js), H], bf16, tag="ot")
                  for jj, j in enumerate(js if do_compute else []):
                      tl, th = int(Tlow[j]), int(Thigh[j])
                      cols = [base + lob + t for t in range(tl)]
                      cols += [base + hib + t for t in range(th)]
                      local = [lob + t for t in range(tl)]
                      local += [hib + t for t in range(th)]
                      lob += tl
                      hib += th
                      pm = pmean.tile([128, BLK], f32)
                      for k, (col, lc) in enumerate(zip(cols, local)):
                          if pre_oh:
                              oh = oh_c[:, lc, :]
                          elif static_oh:
                              oh = oh_const[:]
                          else:
                              oht = ohp.tile([128, BLK], bf16, tag="oh")
                              nc.vector.tensor_scalar(
                                  out=oht[:],
                                  in0=iota_s[:],
                                  scalar1=ridT_s[:, col : col + 1],
                                  scalar2=invcT_s[:, col : col + 1],
                                  op0=mybir.AluOpType.is_equal,
                                  op1=mybir.AluOpType.mult,
                              )
                              oh = oht[:]
                          nc.tensor.matmul(
                              out=pm[:], lhsT=g[:, lc, :], rhs=oh,
                              start=(k == 0), stop=(k == len(cols) - 1),
                          )
                      mean_s = meanp.tile([128, BLK], bf16)
                      if pre_oh:
                          nc.vector.tensor_copy(out=mean_s[:], in_=pm[:])
                      else:
                          nc.scalar.copy(out=mean_s[:], in_=pm[:])
                      po = pout.tile([128, H], f32)
                      nc.tensor.matmul(out=po[:], lhsT=mean_s[:], rhs=wt_s[:],
                                       start=True, stop=False)
                      nc.tensor.matmul(
                          out=po[:], lhsT=selfT_s[:, j * BLK : (j + 1) * BLK],
                          rhs=wb_s[:], start=False, stop=not with_bias,
                      )
                      if with_bias:
                          nc.tensor.matmul(out=po[:], lhsT=ones_s[:],
                                           rhs=bias_s[:], start=False, stop=True)
                      nc.scalar.activation(
                          out=ot_g[:, jj, :], in_=po[:],
                          func=mybir.ActivationFunctionType.Relu,
                      )
                  if do_compute:
                      nc.sync.dma_start(
                          out_d[:, js[0] : js[0] + len(js), :], ot_g[:]
                      )

    mybir.codegen_inst_isa_subclasses(nc)
    _split_excess_waits(nc)
    return nc


def _host_prep(nodes, senders, receivers, W, b, pre_oh=False):
    counts = np.bincount(receivers, minlength=N).astype(np.float32)
    invc_all = (1.0 / np.maximum(counts, 1.0)).astype(np.float32)

    core = receivers // R
    block = (receivers % R) // BLK

    # int16 gather indices force two base views of the node table:
    #   A = nodes[:HALF]          (senders <  HALF)
    #   B = nodes[BOFF:]          (senders >= BOFF), BOFF = N - HALF
    # Senders in [BOFF, HALF) can use either view; assign them per
    # (core, block) so the block's column count hits ceil(total/128).
    BOFF = N - HALF
    ecls = np.where(senders < BOFF, 0, np.where(senders < HALF, 1, 2))

    cb = core * NBLK + block
    cnt3 = np.zeros((NCORES * NBLK, 3), np.int64)
    np.add.at(cnt3, (cb, ecls), 1)
    cnt3 = cnt3.reshape(NCORES, NBLK, 3)
    aa, ff, bb = cnt3[:, :, 0], cnt3[:, :, 1], cnt3[:, :, 2]
    tot = aa + ff + bb

    T = np.maximum(np.ceil(tot / 128.0).astype(np.int64).max(axis=0), 1)
    Tlow = np.ceil(aa / 128.0).astype(np.int64).max(axis=0)
    Thigh_min = np.ceil(bb / 128.0).astype(np.int64).max(axis=0)
    T = np.maximum(T, Tlow + Thigh_min)
    Thigh = T - Tlow

    # flex->A count per (core, block): minimum that keeps B within capacity
    x = np.clip(bb + ff - 128 * Thigh[None, :], 0, None)
    assert (x <= ff).all() and (aa + x <= 128 * Tlow[None, :]).all()

    # per-edge half: class0 -> A, class2 -> B, flex: first x (in stable
    # order within the (core, block) group) -> A, rest -> B
    key3 = cb * 4 + ecls
    order3 = np.argsort(key3, kind="stable")
    k3 = key3[order3]
    c3 = np.bincount(k3, minlength=NCORES * NBLK * 4)
    s3 = np.concatenate([[0], np.cumsum(c3)[:-1]])
    rank3 = np.arange(len(k3)) - s3[k3]  # rank within (cb, cls) group
    half_sorted = np.where(
        k3 % 4 == 0, 0, np.where(k3 % 4 == 2, 1, (rank3 >= x.reshape(-1)[k3 // 4]))
    )
    half = np.empty(len(senders), np.int64)
    half[order3] = half_sorted

    gkey = cb * 2 + half
    order = np.argsort(gkey, kind="stable")
    gk = gkey[order]
    ss = senders[order]
    rr = receivers[order]

    ngroups = NCORES * NBLK * 2
    cnt = np.bincount(gk, minlength=ngroups)
    gstart = np.concatenate([[0], np.cumsum(cnt)[:-1]])
    pos = np.arange(len(gk)) - gstart[gk]
    assert (cnt.reshape(NCORES, NBLK, 2) <= 128 * np.stack([Tlow, Thigh], 1)[None]).all()

    # global column base per (block, half) in chunk order: lows then highs
    col_base = np.zeros((NBLK, 2), np.int64)
    base = 0
    for c0 in range(0, NBLK, CHUNK):
        js = range(c0, min(c0 + CHUNK, NBLK))
        for j in js:
            col_base[j, 0] = base
            base += Tlow[j]
        for j in js:
            col_base[j, 1] = base
            base += Thigh[j]
    NT = int(base)

    # per-edge placement
    e_core = gk // (NBLK * 2)
    e_blockhalf = gk % (NBLK * 2)
    e_block = e_blockhalf // 2
    e_half = e_blockhalf % 2
    e_col = col_base[e_block, e_half] + pos // 128
    e_p = pos % 128

    idx_arr = np.zeros((NCORES, NT, 128), np.int16)
    rid_arr = np.full((NCORES, NT, 128), -1.0, np.float32)
    invc_arr = np.zeros((NCORES, NT, 128), np.float32)
    flat = (e_core * NT + e_col) * 128 + e_p
    idx_arr.reshape(-1)[flat] = (ss - e_half * (N - HALF)).astype(np.int16)
    rid_rel = rr - e_core * R - e_block * BLK
    rid_arr.reshape(-1)[flat] = rid_rel.astype(np.float32)
    invc_arr.reshape(-1)[flat] = invc_all[rr]

    if pre_oh:
        # precomputed one-hots: ohT[c, p(edge slot), col, r] = invc at r=rid
        ohT = np.zeros((NCORES, 128, NT, BLK), NBF)
        ohT[e_core, e_p, e_col, rid_rel] = invc_all[rr].astype(NBF)

    # int16 gather table: [16, NT*8] replicated to 128 partitions;
    # entry for (col, p) at row p%16, column col*8 + p//16
    idx_tab = (
        idx_arr.reshape(NCORES, NT, 8, 16)
        .transpose(0, 3, 1, 2)
        .reshape(NCORES, 16, NT * 8)
    )
    idx_tab = np.tile(idx_tab, (1, 8, 1))  # [C, 128, NT*8]

    ridT = rid_arr.transpose(0, 2, 1)   # [C, 128, NT]
    invcT = invc_arr.transpose(0, 2, 1)

    nodes_bf = np.ascontiguousarray(nodes.astype(NBF))
    wt = np.ascontiguousarray(W[:D, :].astype(NBF))
    wb = np.ascontiguousarray(W[D:, :].astype(NBF))
    bias = np.ascontiguousarray(b.reshape(1, H).astype(NBF))
    with_bias = bool(np.any(b))

    in_maps = []
    for c in range(NCORES):
        selfT = np.zeros((D, RPAD), NBF)
        selfT[:, :R] = nodes_bf[c * R : (c + 1) * R].T
        m = {
            "nodes": nodes_bf,
            "idx": np.ascontiguousarray(idx_tab[c]),
            "selfT": selfT,
            "wt": wt,
            "wb": wb,
        }
        if pre_oh:
            m["ohT"] = np.ascontiguousarray(ohT[c].reshape(128, NT * BLK))
        else:
            m["ridT"] = np.ascontiguousarray(ridT[c])
            m["invcT"] = np.ascontiguousarray(invcT[c])
        if with_bias:
            m["bias"] = bias
        in_maps.append(m)
    return Tlow, Thigh, with_bias, in_maps


def kernel(**inputs):
    nodes = np.asarray(inputs["nodes"], dtype=np.float32)
    senders = np.asarray(inputs["senders"]).astype(np.int64)
    receivers = np.asarray(inputs["receivers"]).astype(np.int64)
    W = np.asarray(inputs["W"], dtype=np.float32)
    b = np.asarray(inputs["b"], dtype=np.float32)

    Tlow, Thigh, with_bias, in_maps = _host_prep(
        nodes, senders, receivers, W, b, pre_oh=True
    )

    key = (tuple(Tlow), tuple(Thigh), with_bias)
    if key not in _prog_cache:
        _prog_cache[key] = _build_program(
            Tlow, Thigh, with_bias, sub_cols=16, pre_oh=True, ohp_bufs=3
        )
    nc = _prog_cache[key]

    res = run_bass_kernel_spmd(nc, in_maps, list(range(NCORES)))
    out = np.concatenate(
        [
            res.results[c]["out"]
            .astype(np.float32)
            .transpose(1, 0, 2)
            .reshape(RPAD, H)[:R]
            for c in range(NCORES)
        ],
        axis=0,
    )
    return out

